# revision 6
# baseline (speedup 1.0000x reference)
"""TRN2 Bass kernel for nn_KW_CascadedBranchPlus (vq_codebook).

Math identity used: the straight-through output
    keywords = (hard + prob - stop_grad(prob)) @ token_emb
is numerically exactly token_emb[argmax_v cos_score] (hard @ token_emb),
since prob - stop_grad(prob) == 0 in the forward pass.
Verified vs the jax reference: max relerr ~1.2e-7 (fp32 noise).

Also: row-normalizing kw does not change argmax over v (positive per-row
scale), so kw normalization is skipped entirely. token_emb normalization
+ transpose happens once on the host (it's a frozen table).

Sharding: vocab-parallel over 8 cores. Each core gets a 6176-row slice of
the 49408-row vocab (padded to 6272 = 49*128 with copies of the slice's
first row — value-safe duplicates), computes scores for all 512 keywords
vs its slice, takes per-keyword max + argmax (vector max/max_index),
gathers the winning embedding rows via indirect DMA, and returns
(candidate rows, max values). The host picks the winning core per keyword
by comparing max values (np.argmax ties break to the lowest core =
lowest global index, matching the reference).

Per-core device pipeline:
  kwT[t,r] = sum_d W[d,t] * audioT[d,r]  (+ b broadcast per-partition)
  scoresT accum over 4 t-chunks: psum[r, v] += kwT[tc][:,rc].T @ embTn[tc][:,v]
  per r-chunk: vector.max (top-8) -> vector.max_index -> indirect gather
"""
import sys

sys.path.insert(0, "/opt/trn_rl_repo")

import numpy as np

import concourse.bass as bass
import concourse.mybir as mybir
from concourse import bacc
import concourse.tile as tile
from concourse.bass_utils import run_bass_kernel_spmd

B, N, D, T, V = 8, 64, 768, 512, 49408
R = B * N            # 512 keywords total
NCORES = 8
VSL = V // NCORES    # 6176 real rows per core
VPAD = 6272          # 49 tiles of 128
NT = T // 128        # 4 t-chunks
NDC = D // 128       # 6 d-chunks
NRC = R // 128       # 4 r-chunks
STW = 512            # supertile width
NST = 13             # 12x512 + 1x128
ST_WIDTHS = [STW] * 12 + [VPAD - 12 * STW]

_CACHE = {}


def _build_bass():
    f32 = mybir.dt.float32
    nc = bacc.Bacc("TRN2", target_bir_lowering=False, debug=True)

    d_audT = nc.dram_tensor("audT", [D, R], f32, kind="ExternalInput")
    d_w = nc.dram_tensor("w", [D, T], f32, kind="ExternalInput")
    d_b = nc.dram_tensor("b", [T], f32, kind="ExternalInput")
    f32r = mybir.dt.float32r
    d_embT = nc.dram_tensor("embT", [T, VPAD], f32r, kind="ExternalInput")
    d_rows = nc.dram_tensor("rows", [VPAD, T], f32, kind="ExternalInput")
    o_cand = nc.dram_tensor("cand", [R, T], f32, kind="ExternalOutput")
    o_maxv = nc.dram_tensor("maxv", [R, 1], f32, kind="ExternalOutput")

    with tile.TileContext(nc) as tc:
        with (
            tc.tile_pool(name="emb", bufs=1) as p_emb,
            tc.tile_pool(name="kw", bufs=1) as p_kw,
            tc.tile_pool(name="proj", bufs=1) as p_proj,
            tc.tile_pool(name="pproj", bufs=2, space="PSUM") as pp_proj,
            tc.tile_pool(name="sc", bufs=2) as p_sc,
            tc.tile_pool(name="psc", bufs=2, space="PSUM") as pp_sc,
            tc.tile_pool(name="small", bufs=1) as p_small,
            tc.tile_pool(name="out", bufs=2) as p_out,
        ):
            # ---- embedding slice DMA: one tile per (t-chunk, supertile) ----
            embt = []
            for tcx in range(NT):
                row = []
                for st in range(NST):
                    w = ST_WIDTHS[st]
                    t_e = p_emb.tile([128, w], f32r, tag=f"emb_{tcx}_{st}")
                    nc.sync.dma_start(
                        t_e[:],
                        d_embT[tcx * 128:(tcx + 1) * 128,
                               st * STW:st * STW + w],
                    )
                    row.append(t_e)
                embt.append(row)

            # ---- bias, strided into [128, NT] (b[tc*128+p] at [p, tc]) ----
            t_b = p_small.tile([128, NT], f32, tag="bias")
            nc.sync.dma_start(
                t_b[:], d_b[:].rearrange("(c p) -> p c", p=128)
            )

            # ---- projection: kwT[t, r] ----
            audt = []
            wt = []
            for dc in range(NDC):
                t_a = p_proj.tile([128, R], f32, tag=f"aud_{dc}")
                nc.sync.dma_start(
                    t_a[:], d_audT[dc * 128:(dc + 1) * 128, :])
                audt.append(t_a)
                t_w = p_proj.tile([128, T], f32, tag=f"w_{dc}")
                nc.sync.dma_start(
                    t_w[:], d_w[dc * 128:(dc + 1) * 128, :])
                wt.append(t_w)

            kwt = []
            for tcx in range(NT):
                p_kwp = pp_proj.tile([128, R], f32, tag="kw_psum")
                for dc in range(NDC):
                    nc.tensor.matmul(
                        p_kwp[:],
                        lhsT=wt[dc][:, tcx * 128:(tcx + 1) * 128],
                        rhs=audt[dc][:],
                        start=(dc == 0),
                        stop=(dc == NDC - 1),
                    )
                t_kw = p_kw.tile([128, R], f32r, tag=f"kwt_{tcx}")
                # evacuate psum + add bias (per-partition scalar)
                nc.vector.tensor_scalar(
                    out=t_kw[:],
                    in0=p_kwp[:],
                    scalar1=t_b[:, tcx:tcx + 1],
                    scalar2=None,
                    op0=mybir.AluOpType.add,
                )
                kwt.append(t_kw)

            # ---- scores + argmax + gather, per r-chunk ----
            for rc in range(NRC):
                t_sc = p_sc.tile([128, VPAD], f32, tag="scores")
                for st in range(NST):
                    w = ST_WIDTHS[st]
                    p_s = pp_sc.tile([128, w], f32, tag="sc_psum")
                    for tcx in range(NT):
                        nc.tensor.matmul(
                            p_s[:],
                            lhsT=kwt[tcx][:, rc * 128:(rc + 1) * 128],
                            rhs=embt[tcx][st][:],
                            start=(tcx == 0),
                            stop=(tcx == NT - 1),
                        )
                    nc.scalar.copy(
                        t_sc[:, st * STW:st * STW + w], p_s[:])

                t_max8 = p_out.tile([128, 8], f32, tag="max8")
                t_idx8 = p_out.tile([128, 8], mybir.dt.uint32, tag="idx8")
                nc.vector.max(out=t_max8[:], in_=t_sc[:])
                nc.vector.max_index(
                    out=t_idx8[:], in_max=t_max8[:], in_values=t_sc[:])

                t_cand = p_out.tile([128, T], f32, tag="cand")
                nc.gpsimd.indirect_dma_start(
                    out=t_cand[:],
                    out_offset=None,
                    in_=d_rows[:],
                    in_offset=bass.IndirectOffsetOnAxis(
                        ap=t_idx8[:, 0:1], axis=0),
                )
                nc.sync.dma_start(
                    o_cand[rc * 128:(rc + 1) * 128, :], t_cand[:])
                nc.sync.dma_start(
                    o_maxv[rc * 128:(rc + 1) * 128, :], t_max8[:, 0:1])
    nc.compile()
    return nc


def kernel(audio_feat, W_proj, b_proj, token_emb):
    audio_feat = np.asarray(audio_feat, dtype=np.float32)
    W_proj = np.asarray(W_proj, dtype=np.float32)
    b_proj = np.asarray(b_proj, dtype=np.float32)
    token_emb = np.asarray(token_emb, dtype=np.float32)

    # host prep: normalize + transpose the frozen table (fp64 for accuracy)
    emb64 = token_emb.astype(np.float64)
    nrm = np.maximum(np.sqrt((emb64 * emb64).sum(-1, keepdims=True)), 1e-8)
    embTn = np.ascontiguousarray((emb64 / nrm).T.astype(np.float32))  # [T, V]
    audT = np.ascontiguousarray(
        audio_feat.reshape(R, D).T).astype(np.float32)  # [D, R]

    in_maps = []
    for c in range(NCORES):
        sl = embTn[:, c * VSL:(c + 1) * VSL]
        slp = np.concatenate(
            [sl, np.repeat(sl[:, :1], VPAD - VSL, axis=1)], axis=1)
        rows = token_emb[c * VSL:(c + 1) * VSL]
        rowsp = np.concatenate(
            [rows, np.repeat(rows[:1], VPAD - VSL, axis=0)], axis=0)
        in_maps.append({
            "audT": audT,
            "w": W_proj,
            "b": b_proj,
            "embT": np.ascontiguousarray(slp),
            "rows": np.ascontiguousarray(rowsp),
        })

    import time as _time
    if "nc" not in _CACHE:
        _CACHE["nc"] = _build_bass()
    _t0 = _time.time()
    r = run_bass_kernel_spmd(
        _CACHE["nc"], in_maps, core_ids=list(range(NCORES)))
    globals()["LAST"] = r
    globals()["LAST_DEVICE_S"] = _time.time() - _t0
    res = r.results

    cands = np.stack([res[c]["cand"] for c in range(NCORES)])   # [8, R, T]
    maxvs = np.stack([res[c]["maxv"][:, 0] for c in range(NCORES)])  # [8, R]
    win = np.argmax(maxvs, axis=0)                              # [R]
    out = cands[win, np.arange(R)]                              # [R, T]
    return out.reshape(B, N, T).astype(np.float32)
